# revision 1
# baseline (speedup 1.0000x reference)
"""Ergodicity loss kernel for Trainium2 (8 NeuronCores, batch-sharded SPMD).

Math: loss = mean((c - coeffs)^2) + REG*sum(u^2)/(2*N*T*B)
      c[b,i,j] = sum_{t,n} cos(i*pi*x0)*cos(j*pi*x1) / (norm[i,j]*N*T)

Device computes, per core (4 of 32 batches, batch-sharded so no collective
is needed; partial results are disjoint):
  - 16 "feature" tiles per spatial dim: fixed linear mixes of cos(k*pi*x_d),
    built from ACT Sin (k=1; args stay within the [-pi/2, pi/2] LUT range),
    ACT Square chains (even k; the 2z-1 affine is absorbed into the next
    activation's scale/bias) and DVE scalar_tensor_tensor / tensor_mul
    (odd k). All features stored bf16, d-interleaved so matmul operands are
    single-stride [[2,128]] APs.
  - C'[b, i, j] = sum_{t,n} F_i(x0) F_j(x1) via accumulating bf16 matmuls
    with 8 n-values packed per matmul (diagonal 16x16 blocks are the real
    per-n products; off-diagonal blocks are junk and ignored), interleaved
    tc-wise with feature computation so PE work hides under ACT/DVE.
  - sum(u^2) on the PE: fp32 Gram-block self-matmuls accumulated into one
    psum tile; the host sums its diagonal.

Host recovers true cos-basis C by inverting the (triangular, well-conditioned)
feature-mixing matrix A, computed here symbolically by replaying the exact
device pipeline in a cos-harmonic algebra, then finishes the loss in float64.

Toolchain notes: this walrus build enforces a 1-sync-wait budget on most
instruction templates, which drives several structural choices (engine
assignment of ops, "opener" matmuls that pre-observe engine semaphores on
PE, one DMA per HW queue sem, and the split kernel-tail drains).
"""

import sys

sys.path.insert(0, "/opt/trn_rl_repo")

import numpy as np

import concourse.bass as bass
import concourse.mybir as mybir
from concourse import bass_utils
from concourse.tile import TileContext
from concourse.tile_rust import add_dep_helper
from concourse.vector_clock import ScopedClock, VectorClock

# This walrus build enforces a small per-instruction sync-wait budget (1 for
# most compute/DMA templates, ~4 for CTRL drains). Tile's kernel-tail barrier
# emits ONE drain waiting on every live proc (engines + DMA queues), which
# exceeds that. Split it into multiple drains of <=3 waits each.
_orig_drain_and_barrier = TileContext._drain_and_barrier


def _split_drain_and_barrier(self, tick_clock, wait_clock):
    gc = tick_clock.global_clock
    ticks = list(gc)
    procs = [i for i, t in enumerate(ticks) if t > 0]
    for p in procs:
        vec = [0] * len(ticks)
        vec[p] = ticks[p]
        d = self.nc.sync.drain()
        wait_clock.add_sem_waits(d.ins, ScopedClock({None: VectorClock(vec)}))
    self.nc.all_engine_barrier(sem_only=True)
    popped = self.nc._tile_sem_poison_stack.pop()
    assert popped is self._sem_poison
    self.nc.clear_and_free_semaphores(list(self.sems.allocated().values()))
    self.nc.all_engine_barrier(sem_only=True)


TileContext._drain_and_barrier = _split_drain_and_barrier

# Problem constants (hardcoded per spec).
K_MAX = 16
N_AGENTS = 64
T = 512
B = 32
D = 2
REG = 1e-3
N_CORES = 8
BPC = B // N_CORES  # batches per core = 4

PI = float(np.pi)

F32 = mybir.dt.float32
BF16 = mybir.dt.bfloat16

# Per-core element geometry: x shard [T=512, BPC=4, N=64, D=2] is host-permuted
# to [128, 2048] with partition p = t % 128 and column (tc, b, n, d),
# tc = t // 128.
TC = 4  # t-chunks of 128
COLS = TC * BPC * N_AGENTS * D  # 2048
HALF = COLS // 2  # columns per slab when split in two (see _body)


# ---------------------------------------------------------------------------
# Symbolic harmonic algebra: every tile value is a fixed linear combination of
# cos(k*pi*x), k = 0..15. We replay the device pipeline here to obtain the
# mixing matrix A (features x harmonics), which the host inverts exactly.
# ---------------------------------------------------------------------------
class Harm:
    __slots__ = ("c",)

    def __init__(self, c):
        self.c = np.asarray(c, dtype=np.float64)

    @staticmethod
    def const(v):
        c = np.zeros(K_MAX)
        c[0] = v
        return Harm(c)

    @staticmethod
    def basis(k, v=1.0):
        c = np.zeros(K_MAX)
        c[k] = v
        return Harm(c)

    def affine(self, scale, bias):
        c = self.c * scale
        c[0] += bias
        return Harm(c)

    def mul(self, other):
        # cos(a)cos(b) = 0.5 cos(a+b) + 0.5 cos(|a-b|)
        out = np.zeros(K_MAX)
        for a in range(K_MAX):
            if self.c[a] == 0.0:
                continue
            for b in range(K_MAX):
                if other.c[b] == 0.0:
                    continue
                v = self.c[a] * other.c[b]
                s, d = a + b, abs(a - b)
                assert s < K_MAX or v == 0.0, f"harmonic overflow {a}+{b}"
                out[s] += 0.5 * v
                out[d] += 0.5 * v
        return Harm(out)

    def square(self, scale=1.0, bias=0.0):
        z = self.affine(scale, bias)
        return z.mul(z)

    def sub(self, other):
        return Harm(self.c - other.c)


def _feature_mixing_matrix():
    """Replay the device feature pipeline symbolically -> A[16,16].

    Must mirror the ops in _body exactly. No tensor_scalar/STT ops are used
    on-device (the STT hardware template carries only one sync wait, which
    the Tile scheduler can exceed), so odd features are plain products and
    clean odd factors come from subtracts of scaled copies of g1.
    """
    f = [None] * K_MAX
    f[0] = Harm.const(1.0)
    g1 = Harm.basis(1, -1.0)  # Sin(pi*x - pi/2) = -cos(pi*x)
    f[1] = g1
    f[2] = g1.square()  # (c2+1)/2
    f[4] = f[2].square(2.0, -1.0)  # (c4+1)/2
    f[8] = f[4].square(2.0, -1.0)  # (c8+1)/2
    f[3] = f[2].affine(1.0, -0.75).mul(f[1])  # stt -> clean c3/4 (signed)
    f[6] = f[3].square(4.0, 0.0)  # (c6+1)/2
    f[12] = f[6].square(2.0, -1.0)  # (c12+1)/2
    f[5] = f[4].affine(1.0, -0.5).mul(f[1])  # stt -> (c5+c3)/4
    f[10] = f[5].mul(f[5])
    f[7] = f[6].affine(1.0, -0.5).mul(f[1])  # stt -> (c7+c5)/4
    f[14] = f[7].mul(f[7])
    f[9] = f[8].mul(f[1])
    f[11] = f[10].mul(f[1])
    f[13] = f[12].mul(f[1])
    f[15] = f[14].mul(f[1])
    A = np.stack([x.c for x in f])
    return A


_A = _feature_mixing_matrix()
_AINV = np.linalg.inv(_A)
assert np.linalg.cond(_A) < 1e4, np.linalg.cond(_A)


def _np_constants():
    """numpy copy of reference._constants() for L=(1,1)."""
    ks = np.arange(K_MAX, dtype=np.float64)
    # integral of exp(i k pi x) over [0,1] -> real part is 1 at k=0 else 0,
    # but compute faithfully like the reference (complex formula).
    vs = []
    for _ in range(D):
        with np.errstate(divide="ignore", invalid="ignore"):
            ki = ks * np.pi
            nz = (np.exp(1j * ki) - 1.0) / (1j * ki)
        integral = np.where(ks == 0, 1.0 + 0j, nz)
        vs.append(integral)
    cd = np.real(vs[0][:, None] * vs[1][None, :]).astype(np.float64)
    norm_last = np.where(ks == 0, 1.0, np.sqrt(0.5))
    norm = np.broadcast_to(norm_last[None, :], (K_MAX, K_MAX)).copy()
    return cd / norm, norm


_COEFFS, _NORM = _np_constants()


# ---------------------------------------------------------------------------
# Device program
# ---------------------------------------------------------------------------
def _body(nc, tc, xu_in, out_dram):
    Sq = mybir.ActivationFunctionType.Square
    Sin = mybir.ActivationFunctionType.Sin
    sub = mybir.AluOpType.subtract
    mult = mybir.AluOpType.mult

    with (
        tc.tile_pool(name="io", bufs=1) as io_pool,
        tc.tile_pool(name="feat", bufs=1) as feat_pool,
        tc.tile_pool(name="work", bufs=1) as work_pool,
        tc.tile_pool(name="psum", bufs=1, space="PSUM") as psum_pool,
    ):
        xu = io_pool.tile([128, 2 * COLS], F32, tag="xu")
        # Input streamed as 7 parallel chunk DMAs (8 HW queue sems total,
        # incl. the output DMA — each DMA gets its own proc, keeping every
        # DMA at <=1 sync wait). x: 4 chunks; u: 3 chunks.
        QC = COLS // 4  # 512-col chunks
        for ci in range(4):
            nc.sync.dma_start(
                out=xu[:, ci * QC : (ci + 1) * QC],
                in_=xu_in[:, ci * QC : (ci + 1) * QC],
            )
        ubnd = [COLS, COLS + 768, COLS + 1408, 2 * COLS]
        for ci in range(3):
            nc.sync.dma_start(
                out=xu[:, ubnd[ci] : ubnd[ci + 1]],
                in_=xu_in[:, ubnd[ci] : ubnd[ci + 1]],
            )
        raw = xu[:, 0:COLS]
        uraw = xu[:, COLS : 2 * COLS]

        # u^2 Gram psum (filled by PE between the two feature-MM slabs).
        psu = psum_pool.tile([128, 128], F32, tag="psu")

        # --- features ---
        # One big bf16 allocation. Column order: (pos, k, d) with
        # pos = (tc, b, no, nl) [1024 values], k = feature [16], d = dim [2].
        # Feature ops address [[32, npos], [1, 2]] (packed d-pairs keep DVE 2x);
        # matmul operands for (tc, b, no) are single-stride [[2, 128]] slices
        # over (nl, k) at d=0 (lhsT) / d=1 (rhs).
        NPOS = COLS // D  # 1024
        FA = feat_pool.tile([128, K_MAX * COLS], BF16, tag="FA")
        FAk = FA[:].rearrange("p (pos k d) -> p k pos d", pos=NPOS, k=K_MAX, d=D)

        def F(k, sl=None):
            if sl is None:
                return FAk[:, k]
            a, b = sl
            return FAk[:, k, a:b]

        g1 = work_pool.tile([128, COLS], F32, tag="g1")
        g1v = g1[:].rearrange("p (pos d) -> p pos d", d=D)
        rawv = raw.rearrange("p (pos d) -> p pos d", d=D)

        Cp = mybir.ActivationFunctionType.Copy
        # Constant feature f0 = 1 on DVE (no deps; DVE has slack while ACT
        # is the dense pole, and matmuls already carry the DVE wait).
        nc.vector.memset(F(0), 1.0)

        # Split into two position slabs so ACT/DVE/Pool work on independent
        # halves and overlap across the dependency chain. Mirror of
        # _feature_mixing_matrix — keep in sync!
        #
        # Wait-slot discipline (most instruction templates carry ONE sync
        # wait): every DVE/Pool op must have at most one "unobserved"
        # producer engine. f1 lives on ACT so mul(F_even, F1) ops have
        # ACT-only inputs + fresh destinations; Pool observes ACT via its
        # first mul, then later Pool ops may carry one DVE wait.
        HP = NPOS // 2
        SL = {0: (0, HP), 1: (HP, NPOS)}

        def act(out, in_, func, **kw):
            nc.scalar.activation(out, in_, func, **kw)

        def sin_pair(si):
            # One Sin per 256-pos span = one input-DMA chunk each (keeps
            # every ACT op at a single DMA sync wait).
            s0, s1 = SL[si]
            for a in range(s0, s1, NPOS // 4):
                b = a + NPOS // 4
                act(g1v[:, a:b], rawv[:, a:b], Sin, scale=PI, bias=-PI / 2)

        # Per-slab issue order (slab 0 fully, then slab 1): keeps the
        # slab-0 matmul stream fed as early as possible; slab-1's ACT work
        # overlaps slab-0's DVE tail naturally.
        for si in (0, 1):
            sl = SL[si]
            s0, s1 = sl
            g1s = g1v[:, s0:s1]
            sin_pair(si)
            act(F(1, sl), g1s, Cp)
            act(F(2, sl), g1s, Sq)
            act(F(4, sl), F(2, sl), Sq, scale=2.0, bias=-1.0)
            act(F(8, sl), F(4, sl), Sq, scale=2.0, bias=-1.0)
            nc.vector.scalar_tensor_tensor(F(3, sl), F(2, sl), 0.75, F(1, sl), sub, mult)
            act(F(6, sl), F(3, sl), Sq, scale=4.0)
            act(F(12, sl), F(6, sl), Sq, scale=2.0, bias=-1.0)
            nc.vector.scalar_tensor_tensor(F(5, sl), F(4, sl), 0.5, F(1, sl), sub, mult)
            nc.vector.tensor_mul(out=F(10, sl), in0=F(5, sl), in1=F(5, sl))
            nc.vector.scalar_tensor_tensor(F(7, sl), F(6, sl), 0.5, F(1, sl), sub, mult)
            nc.vector.tensor_mul(out=F(14, sl), in0=F(7, sl), in1=F(7, sl))
            if si == 0:
                nc.vector.tensor_mul(out=F(9, sl), in0=F(8, sl), in1=F(1, sl))
                nc.vector.tensor_mul(out=F(13, sl), in0=F(12, sl), in1=F(1, sl))
                nc.vector.tensor_mul(out=F(11, sl), in0=F(10, sl), in1=F(1, sl))
                nc.vector.tensor_mul(out=F(15, sl), in0=F(14, sl), in1=F(1, sl))
            else:
                # Split slab-1 leaves per tc so the tc2 matmuls' inputs are
                # complete a few us before tc3's (byte-exact deps let the PE
                # start the tc2 burst early, halving the exposed MM tail).
                mid2 = (s0 + s1) // 2
                for ha, hb in ((s0, mid2), (mid2, s1)):
                    hs = (ha, hb)
                    nc.vector.tensor_mul(out=F(9, hs), in0=F(8, hs), in1=F(1, hs))
                    nc.vector.tensor_mul(out=F(13, hs), in0=F(12, hs), in1=F(1, hs))
                    nc.vector.tensor_mul(out=F(11, hs), in0=F(10, hs), in1=F(1, hs))
                    nc.vector.tensor_mul(out=F(15, hs), in0=F(14, hs), in1=F(1, hs))

        # --- matmuls ---
        # lhsT/rhs for (tc, b, octet): [[2, 128]] over (nl, k) at d=0 / d=1.
        FAv = FA[:].rearrange(
            "p (tb no ck d) -> p tb no d ck",
            tb=TC * BPC, no=8, ck=128, d=D,
        )
        csb = work_pool.tile([128, (BPC + 1) * 128], F32, tag="csb")
        pstiles = [
            psum_pool.tile([128, 128], F32, tag=f"ps{b}", name=f"ps{b}")
            for b in range(BPC)
        ]

        # Openers per slab: tiny matmuls reading the slab's LAST ACT-written
        # feature (f12) and LAST Pool-written feature (f15). They absorb the
        # ACT/Pool waits on the PE sequencer, so every real Ldweights carries
        # at most the single DVE wait its template allows. Their outputs land
        # in off-diagonal (junk) cells of the u^2 Gram psum — live but unread.
        # Matmuls for tc-chunks {0,1} only need slab 0's features, so they
        # run concurrently with slab 1's feature computation (interleaved
        # PSUM accumulation groups, one per b).
        last_mm = None
        for si, (s0, s1) in enumerate((SL[0], SL[1])):
            openers = []
            for oi, fk in enumerate((12, 15)):
                op = nc.tensor.matmul(
                    psu[0:2, 8 + 4 * oi : 10 + 4 * oi],
                    F(fk, (s1 - 1, s1)), F(fk, (s1 - 1, s1)),
                    start=True, stop=True, skip_group_check=True,
                )
                if last_mm is not None:
                    # Don't let slab-1 openers jump ahead of slab-0 matmuls
                    # (they'd stall PE on slab-1 feature completion).
                    add_dep_helper(op.ins, last_mm.ins, sync=False,
                                   reason="opener after prev slab MMs")
                openers.append(op)
            for tci in (2 * si, 2 * si + 1):
                for b in range(BPC):
                    ps = pstiles[b]
                    for oc in range(8):
                        tb = tci * BPC + b
                        mm = nc.tensor.matmul(
                            ps[:], FAv[:, tb, oc, 0], FAv[:, tb, oc, 1],
                            start=(tci == 0 and oc == 0),
                            stop=(tci == TC - 1 and oc == 7),
                            skip_group_check=True,
                        )
                        for op in openers:
                            add_dep_helper(mm.ins, op.ins, sync=False,
                                           reason="PE wait-slot opener")
                        last_mm = mm
            if si == 0:
                # u^2 Gram: 16 fp32 self-matmuls accumulating u_blk^T@u_blk;
                # host sums the psum diagonal. Placed between the two
                # feature-MM bursts: it fills the PE gap while slab-1
                # features are still computing.
                for c in range(16):
                    ub = uraw[:, c * 128 : (c + 1) * 128]
                    um = nc.tensor.matmul(
                        psu[:], ub, ub, start=(c == 0), stop=(c == 15),
                        skip_group_check=True,
                    )
                    add_dep_helper(um.ins, last_mm.ins, sync=False,
                                   reason="u-gram after slab-0 MMs")

        # PSUM -> SBUF on ACT (single-engine csb producers keep the output
        # DMA at one sync wait; ScalarE has a fast PSUM read port).
        for b in range(BPC):
            nc.scalar.copy(out=csb[:, b * 128 : (b + 1) * 128], in_=pstiles[b][:])
        nc.scalar.copy(out=csb[:, BPC * 128 : (BPC + 1) * 128], in_=psu[:])
        nc.sync.dma_start(out=out_dram[:], in_=csb[:])


_CACHE = {}


def _register_const(nc, value, dtype=F32):
    t = nc.alloc_sbuf_tensor(f"const-{dtype.name}-{value}", [128, 1], dtype)
    nc.gpsimd.memset(t.ap(), value)
    nc.const_aps.aps[(dtype, value)] = t.ap()


def _build():
    if "nc" in _CACHE:
        return _CACHE["nc"]
    nc = bass.Bass("TRN2", debug=False)
    # The kernel-tail semaphore cleanup calls gpsimd.dma_reset (a DGE-queue
    # drain, ~3-4us). All DMAs are already completion-waited by the split
    # drains, and no dynamic DMA state is used, so skip it. Re-execution
    # correctness is validated by the harness (second run checked).
    type(nc.gpsimd).dma_reset = lambda self, semaphore_range=None: None
    _register_const(nc, -PI / 2)
    _register_const(nc, -1.0)
    nc.all_engine_barrier()
    xu_in = nc.dram_tensor("xu", [128, 2 * COLS], F32, kind="ExternalInput")
    out_d = nc.dram_tensor("out", [128, (BPC + 1) * 128], F32, kind="ExternalOutput")
    with TileContext(nc) as t:
        _body(nc, t, xu_in.ap(), out_d.ap())
    _CACHE["nc"] = nc
    return nc


def _shard_host(a):
    """[T, B, N, D] -> per-core [128, COLS] t-major layout."""
    out = []
    for c in range(N_CORES):
        s = a[:, c * BPC : (c + 1) * BPC]  # [512, 4, 64, 2]
        s = s.reshape(TC, 128, BPC, N_AGENTS, D)  # (tc, p, b, n, d)
        s = np.ascontiguousarray(np.transpose(s, (1, 0, 2, 3, 4)))
        out.append(s.reshape(128, COLS))
    return out


def kernel(x, u, **_):
    x = np.asarray(x, dtype=np.float32)
    u = np.asarray(u, dtype=np.float32)
    nc = _build()
    xs = _shard_host(x)
    us = _shard_host(u)
    in_maps = [
        {"xu": np.ascontiguousarray(np.concatenate([xs[c], us[c]], axis=1))}
        for c in range(N_CORES)
    ]
    res = bass_utils.run_bass_kernel_spmd(nc, in_maps, core_ids=list(range(N_CORES)))
    return _finish_host(res.results)


def _finish_host(outs):
    """Host reduction/unmixing in float64 -> scalar loss."""
    Cp = np.zeros((B, K_MAX, K_MAX), dtype=np.float64)
    u2 = 0.0
    for c in range(N_CORES):
        o = outs[c]["out"].astype(np.float64)  # [128, 640]
        craw = o[:, : BPC * 128]
        ublk = o[:, BPC * 128 : (BPC + 1) * 128]
        u2 += float(np.trace(ublk))
        for b in range(BPC):
            blk = craw[:, b * 128 : (b + 1) * 128]
            acc = np.zeros((K_MAX, K_MAX))
            for nl in range(8):
                acc += blk[nl * 16 : nl * 16 + 16, nl * 16 : nl * 16 + 16]
            Cp[c * BPC + b] = acc

    # C' = A C_true A^T  (same A both dims) -> C_true = Ainv C' Ainv^T
    Ct = np.einsum("ik,bkl,jl->bij", _AINV, Cp, _AINV)
    c = Ct / (_NORM[None] * (N_AGENTS * T))
    loss = np.mean((c - _COEFFS[None]) ** 2)
    loss = loss + REG * u2 / (2.0 * N_AGENTS * T * B)
    return np.array(loss, dtype=np.float32)


if __name__ == "__main__":
    rng = np.random.default_rng(0)
    x = rng.random((T, B, N_AGENTS, D), dtype=np.float32)
    u = rng.standard_normal((T, B, N_AGENTS, D)).astype(np.float32)
    print(kernel(x=x, u=u))

